# revision 1
# baseline (speedup 1.0000x reference)
"""Trainium2 Bass kernel for the MsaHmmCell forward scan.

Problem: HMM forward algorithm, M=2 models x B=64 sequences, T=512 steps,
q=515 states, D=26 obs dims. Output = log unnormalized forward variables
[T, M, B, q] (float32).

Strategy (8 NeuronCores, SPMD - one program, per-core data):
  core k -> (model m = k // 4, time chunk c = k % 4), chunks of 128 steps.
  Each core runs TWO INTERLEAVED half-chunk scans (64 outputs each) so one
  scan's matmuls hide the other's recurrence latency.

  The scan is UNNORMALIZED: since EPS=1e-32 is negligible,
  out_t = log(alpha_hat_t) with alpha_hat_{t+1} = E_{t+1} o (alpha_hat_t @ A).
  fp16 dynamic range is held by folding a 2x rescale into every E slab
  (2^10 into slab 0); the deterministic log-offset is subtracted on host.

  Each half-chunk scan starts W=8 steps early from an arbitrary init
  (E o pi); the nearly-uniform transition matrix mixes in a few steps, so
  the normalized direction converges to machine precision. The unknown
  per-(m,b) scale gamma of each half-chunk is recovered on the host by
  matching row sums at boundaries (each scan runs one step past its end
  and emits start/end row sums). t=0..2 are computed exactly on the host.

  Device layout: alpha_hat TRANSPOSED [q (5x128 chunks), (scan, b)] fp16.
  Per step: 25 matmuls (A chunks stationary as weights, alphaT moving with
  N=128 covering BOTH scans' batch columns) accumulate R^T into two PSUM
  tiles; DVE multiplies by E^T slabs (SBUF-resident; E = BmT @ obsT matmuls
  are emitted interleaved with the scan so the scheduler fills recurrence
  -latency gaps). Output: each state pair is PE-transposed (vs identity)
  into a fp16 PSUM tile as [(t,b), (scan, q)]; ScalarE applies Ln reading
  PSUM directly -> fp32 SBUF; one DMA per pair writes four [64, 515]
  output slabs. Per-core outputs: "out" [128, 64, 515] and "bsums" [4, 64]
  (start/end row sums per scan for the host gamma chain).
"""

import sys

sys.path.insert(0, "/opt/trn_rl_repo")

import numpy as np

# ---------------- problem constants (hardcoded per contract) ----------------
M, B, T, D = 2, 64, 512, 26
Q = 515
QPAD = 640
KC = 5  # q chunks of 128
W = 8  # warmup steps
NSCAN = 2  # interleaved half-chunk scans per core
HALF = 64  # output steps per scan
NJS = W + HALF + 2  # states per scan: W warmup, 64 outputs, boundary, dummy
CHUNK = 128
NCORES = 8
LN2 = float(np.log(2.0))
INIT_EXP = 10  # slab-0 scale 2^10
HOST_EXACT = 3  # first outputs computed exactly on host

_prog_cache = {}


def _softmax(x, axis=-1):
    x = x.astype(np.float64)
    m = x.max(axis=axis, keepdims=True)
    e = np.exp(x - m)
    return (e / e.sum(axis=axis, keepdims=True)).astype(np.float32)


def _build_program():
    import concourse.tile as tile
    from concourse import bacc, mybir
    from contextlib import ExitStack

    f16 = mybir.dt.float16
    f32 = mybir.dt.float32
    Ln = mybir.ActivationFunctionType.Ln
    NBT = NSCAN * NJS * B  # obsT / per-kc E free size

    nc = bacc.Bacc(
        "TRN2",
        debug=False,
        enable_asserts=False,
        target_bir_lowering=False,
        num_devices=NCORES,
    )

    obst_d = nc.dram_tensor("obst", [D, NBT], f16, kind="ExternalInput").ap()
    a_d = nc.dram_tensor("a_t", [QPAD, QPAD], f16, kind="ExternalInput").ap()
    bmt_d = nc.dram_tensor("bmt", [D, QPAD], f16, kind="ExternalInput").ap()
    pit_d = nc.dram_tensor("pit", [128, KC], f32, kind="ExternalInput").ap()
    id_d = nc.dram_tensor("ident", [128, 128], f16, kind="ExternalInput").ap()
    out_d = nc.dram_tensor("out", [CHUNK, B, Q], f32, kind="ExternalOutput").ap()
    bs_d = nc.dram_tensor("bsums", [2 * NSCAN, B], f32, kind="ExternalOutput").ap()


    with tile.TileContext(nc) as tc:
        with ExitStack() as ctx:
            const = ctx.enter_context(tc.tile_pool(name="const", bufs=1))
            stage_p = ctx.enter_context(tc.tile_pool(name="stage", bufs=12))
            fmt_p = ctx.enter_context(tc.tile_pool(name="fmt", bufs=1, space="PSUM"))
            outst_p = ctx.enter_context(tc.tile_pool(name="outst", bufs=8))
            bs_p = ctx.enter_context(tc.tile_pool(name="bs", bufs=2))

            # ---- persistent tiles ----
            obst = const.tile([D, NBT], f16, tag="obst")
            nc.sync.dma_start(obst[:], obst_d[:])
            bmt = const.tile([D, QPAD], f16, tag="bmt")
            nc.sync.dma_start(bmt[:], bmt_d[:])
            pit = const.tile([128, KC], f32, tag="pit")
            nc.sync.dma_start(pit[:], pit_d[:])
            ident = const.tile([128, 128], f16, tag="ident")
            nc.sync.dma_start(ident[:], id_d[:])
            a_sb = []
            for k in range(KC):
                t = const.tile([128, QPAD], f16, tag=f"a{k}", name=f"a{k}")
                nc.sync.dma_start(t[:], a_d[128 * k : 128 * (k + 1), :])
                a_sb.append(t)
            e_all = const.tile([128, KC * NBT], f16, tag="e_all")
            # [128, kc, j*NSCAN+s, b]
            e_v = e_all.rearrange("p (k t u) -> p k t u", k=KC, u=B)

            # ---- phase 1: E^T precompute (chunks emitted interleaved with
            # the scan so the scheduler can fill recurrence-latency gaps) ----
            epool = ctx.enter_context(tc.tile_pool(name="epsum", bufs=2, space="PSUM"))
            nch = (NBT + 511) // 512

            def emit_e_chunk(ci):
                for k in range(KC):
                    w = min(512, NBT - ci * 512)
                    ps = epool.tile([128, 512], f32, tag="eps", bufs=2,
                                    name=f"eps{ci}_{k}")
                    nc.tensor.matmul(
                        ps[:, :w],
                        lhsT=bmt[:, 128 * k : 128 * (k + 1)],
                        rhs=obst[:, ci * 512 : ci * 512 + w],
                        start=True,
                        stop=True,
                    )
                    dst = e_all[:, k * NBT + ci * 512 : k * NBT + ci * 512 + w]
                    if (k * nch + ci) % 2 == 0:
                        nc.scalar.copy(dst, ps[:, :w])
                    else:
                        nc.vector.tensor_copy(dst, ps[:, :w])

            # chunks 0-1 up front (init + first scan steps)
            next_ci = 2
            emit_e_chunk(0)
            emit_e_chunk(1)

            # ---- phase 2: two interleaved scans ----
            spsum = ctx.enter_context(tc.tile_pool(name="spsum", bufs=2, space="PSUM"))

            def emit_fmt(p, cur):
                """PE-transpose pair p (both scans) into PSUM, log, store."""
                fmt = fmt_p.tile([128, NSCAN * QPAD], f16, tag="fmt")
                for blk in range(NSCAN * KC):
                    nc.tensor.transpose(
                        fmt[:, 128 * blk : 128 * (blk + 1)],
                        cur[:, 128 * blk : 128 * (blk + 1)],
                        ident[:],
                    )
                fv = fmt.rearrange("p (s q) -> p s q", s=NSCAN)
                jj = 2 * p
                if W <= jj < W + HALF:
                    outst = outst_p.tile([128, NSCAN * Q], f32, tag="outst")
                    ov = outst.rearrange("p (s q) -> p s q", s=NSCAN)
                    nc.scalar.activation(ov[:], fv[:, :, 0:Q], Ln)
                    # partitions (ts, b); free (s, q); dest t = s*HALF + jj-W + ts
                    t0 = jj - W
                    # rows of outst map to (ts, b) -> out_d[t0+ts] rows
                    nc.scalar.dma_start(
                        out_d.rearrange("(s2 t) b q -> t b s2 q", s2=NSCAN)[
                            t0 : t0 + 2
                        ].rearrange("t b s2 q -> (t b) s2 q"),
                        ov[:],
                    )
                if jj == W or jj == W + HALF:
                    ix = 0 if jj == W else 1
                    for s in range(NSCAN):
                        bs = bs_p.tile([64, 1], f32, tag="bs", name=f"bs{p}_{s}")
                        nc.vector.reduce_sum(
                            bs[:], fv[0:64, s, 0:Q], axis=mybir.AxisListType.X
                        )
                        nc.sync.dma_start(bs_d[2 * s + ix], bs[:])

            # init states j=0 for both scans; stage tile [128, (s, kc, sl, b)]
            cur = stage_p.tile([128, NSCAN * QPAD], f16, tag="stage", name="st0")
            for s in range(NSCAN):
                for pc in range(KC):
                    nc.scalar.mul(
                        cur[:, QPAD * s + 128 * pc : QPAD * s + 128 * pc + 64],
                        e_v[:, pc, s, :],
                        pit[:, pc : pc + 1],
                    )

            for j in range(1, NJS):
                # scan step j consumes E chunk floor((2j+1)*64/512); keep a
                # 2-chunk lead emitted just-in-time
                while next_ci < nch and next_ci <= (2 * j + 2 * 2) // 8 + 2:
                    emit_e_chunk(next_ci)
                    next_ci += 1
                p, sl = j // 2, j % 2
                psl = (j - 1) % 2
                prev = cur
                if sl == 0:
                    cur = stage_p.tile(
                        [128, NSCAN * QPAD], f16, tag="stage", name=f"st{j}"
                    )
                # [p, s, kc, u] views
                cv = cur.rearrange("p (s k u) -> p s k u", s=NSCAN, k=KC)
                pv = prev.rearrange("p (s k u) -> p s k u", s=NSCAN, k=KC)
                psa = spsum.tile([128, 384], f32, tag="psa", bufs=2, name=f"psa{j}")
                psb = spsum.tile([128, 256], f32, tag="psb", bufs=2, name=f"psb{j}")
                pav = psa.rearrange("p (k s u) -> p k s u", k=3, s=NSCAN)
                pbv = psb.rearrange("p (k s u) -> p k s u", k=2, s=NSCAN)
                for pc in range(KC):
                    dst = pav[:, pc, :, :] if pc < 3 else pbv[:, pc - 3, :, :]
                    for k in range(KC):
                        nc.tensor.matmul(
                            dst,
                            lhsT=a_sb[k][:, 128 * pc : 128 * (pc + 1)],
                            rhs=pv[:, :, k, 64 * psl : 64 * psl + 64],
                            start=(k == 0),
                            stop=(k == KC - 1),
                        )
                    if pc == 2:
                        nc.vector.tensor_mul(
                            cv[:, :, 0:3, 64 * sl : 64 * (sl + 1)].rearrange(
                                "p s k u -> p k s u"
                            ),
                            pav[:],
                            e_v[:, 0:3, NSCAN * j : NSCAN * j + NSCAN, :],
                        )
                nc.vector.tensor_mul(
                    cv[:, :, 3:KC, 64 * sl : 64 * (sl + 1)].rearrange(
                        "p s k u -> p k s u"
                    ),
                    pbv[:],
                    e_v[:, 3:KC, NSCAN * j : NSCAN * j + NSCAN, :],
                )
                if sl == 1 and p >= W // 2:
                    emit_fmt(p, cur)
            while next_ci < nch:
                emit_e_chunk(next_ci)
                next_ci += 1

    nc.compile()
    return nc


def _host_prep(inputs):
    obs = np.asarray(inputs["obs"], np.float32)
    A = _softmax(np.asarray(inputs["A_logits"]))
    Bm = _softmax(np.asarray(inputs["B_logits"]))
    pi = _softmax(np.asarray(inputs["init_logits"]))

    A_pad = np.zeros((M, QPAD, QPAD), np.float32)
    A_pad[:, :Q, :Q] = A
    BmT_pad = np.zeros((M, D, QPAD), np.float32)
    BmT_pad[:, :, :Q] = Bm.transpose(0, 2, 1)
    pi_pad = np.zeros((M, QPAD), np.float32)
    pi_pad[:, :Q] = pi
    piT = pi_pad.reshape(M, KC, 128).transpose(0, 2, 1).copy()  # [M, 128, KC]

    slab_scale = np.full(NJS, 2.0, np.float32)
    slab_scale[0] = float(2.0**INIT_EXP)

    in_maps = []
    for core in range(NCORES):
        m, c = core // 4, core % 4
        obsT = np.empty((D, NJS, NSCAN, B), np.float16)
        for s in range(NSCAN):
            ts = np.clip(c * CHUNK + s * HALF - W + np.arange(NJS), 0, T - 1)
            ow = obs[m][:, ts, :] * slab_scale[None, :, None]  # [B, NJS, D]
            obsT[:, :, s] = ow.transpose(2, 1, 0).astype(np.float16)
        in_maps.append(
            {
                "obst": np.ascontiguousarray(obsT).reshape(D, NSCAN * NJS * B),
                "a_t": A_pad[m].astype(np.float16),
                "bmt": BmT_pad[m].astype(np.float16),
                "pit": piT[m].astype(np.float32),
                "ident": np.eye(128, dtype=np.float16),
            }
        )
    return in_maps, (obs, A, Bm, pi)


def _host_assemble(results, obs, A, Bm, pi):
    out = np.empty((T, M, B, Q), np.float32)
    E0 = obs[:, :, 0, :] @ Bm.transpose(0, 2, 1)  # [M, B, Q]
    a0 = E0 * pi[:, None, :]
    true0 = np.log(a0.sum(-1))  # [M, B]

    wconst = LN2 * (INIT_EXP + W + np.arange(HALF, dtype=np.float32))  # [64]
    sW = LN2 * (INIT_EXP + W)

    for m in range(M):
        lng = None
        for c in range(4):
            r = results[m * 4 + c]
            bsums = np.asarray(r["bsums"], np.float64)
            rout = np.asarray(r["out"], np.float32)
            for s in range(NSCAN):
                bss = bsums[2 * s]
                if c == 0 and s == 0:
                    lng = np.log(bss) - sW - true0[m]
                else:
                    prev = (
                        np.asarray(results[m * 4 + c - 1]["bsums"], np.float64)[3]
                        if s == 0
                        else bsums[1]
                    )
                    lng = lng + np.log(bss) - np.log(prev) + HALF * LN2
                t0 = c * CHUNK + s * HALF
                out[t0 : t0 + HALF, m] = (
                    rout[s * HALF : (s + 1) * HALF]
                    - wconst[:, None, None]
                    - lng[None, :, None].astype(np.float32)
                )

    # exact first steps on host (chunk-0 warmup has no pre-t=0 data)
    a = a0.astype(np.float64)
    ll = np.zeros((M, B, 1))
    for t in range(HOST_EXACT):
        S = a.sum(-1, keepdims=True)
        ll = ll + np.log(S)
        a = a / S
        out[t] = (np.log(a + 1e-32) + ll).astype(np.float32)
        Et1 = obs[:, :, t + 1, :].astype(np.float64) @ Bm.transpose(0, 2, 1)
        a = Et1 * np.einsum("mbq,mqp->mbp", a, A)
    return out


def kernel(**inputs) -> np.ndarray:
    from concourse import bass_utils

    in_maps, host_data = _host_prep(inputs)

    if "nc" not in _prog_cache:
        _prog_cache["nc"] = _build_program()
    nc = _prog_cache["nc"]

    res = bass_utils.run_bass_kernel_spmd(nc, in_maps, core_ids=list(range(NCORES)))
    return _host_assemble(res.results, *host_data)


if __name__ == "__main__":
    rng = np.random.default_rng(0)
    ins = {
        "obs": rng.random((M, B, T, D), np.float32),
        "A_logits": (rng.standard_normal((M, Q, Q)) * 0.1).astype(np.float32),
        "B_logits": (rng.standard_normal((M, Q, D)) * 0.1).astype(np.float32),
        "init_logits": (rng.standard_normal((M, Q)) * 0.1).astype(np.float32),
    }
    o = kernel(**ins)
    print("out", o.shape, o.dtype, np.isfinite(o).all())



# revision 26
# speedup vs baseline: 2.2517x; 2.2517x over previous
"""Trainium2 Bass kernel for the MsaHmmCell forward scan (fp8 DoubleRow).

Problem: HMM forward algorithm, M=2 models x B=64 sequences, T=512 steps,
q=515 states, D=26 obs dims. Output = log unnormalized forward variables
[T, M, B, q] (float32).

Strategy (8 NeuronCores, SPMD): core k -> (model m = k // 4, time chunk
c = k % 4) of 128 steps, split into NSCAN=3 interleaved (anti-phased)
scans of ~43 outputs each. The device runs a PURE fp8 scan:

  hat_{j+1} = (hat_j @ A') (*) E'_j * 2^-10      (per scan, per step)

  - A' = A * 2^6 as fp8e4, DoubleRow matmuls: states chunked 515 = 5*103
    (no ragged tail); 15 MMs x N=64 per scan-step, all K via 206-pairs.
  - E' = 32 * obs_t.Bm^T / (2*mean_d obs) is precomputed ON HOST (2.6% of
    the FLOPs, same class as the host softmax prep), quantized to fp8 and
    streamed in; the obs normalization keeps the per-step growth at
    exactly 2x so fp8 never over/underflows.
  - The elementwise stage update is split into two independent batch-
    column lanes so three engines share it: cols [0, XS) PE->DVE
    (scalar_tensor_tensor reading R-PSUM); cols [XS, 64) PE->ACT
    (R*2^-10 -> fp8 SBUF) -> GPSIMD (SBUF-only multiply).
  - Raw fp8 stage slots are DMA'd to DRAM in 4-step blocks; the host
    decodes fp8 via a 256-entry LUT, takes logs, transposes, recovers
    per-scan scale factors gamma from start/end column sums, and adds
    back the deterministic normalizers in f64.
"""

import sys

sys.path.insert(0, "/opt/trn_rl_repo")

import numpy as np
import ml_dtypes

# ---------------- problem constants (hardcoded per contract) ----------------
M, B, T, D = 2, 64, 512, 26
Q = 515
PC = 103          # states per out-chunk (5 * 103 = 515, no tail)
NPC = 5
KKC = 3           # DR contraction chunks (206, 206, 103+zero-pair)
MPAD = 528        # A tile free width (pair stride, % 16 == 0)
NSCAN = 3
W = 4             # warmup steps (fp8 noise floor ~1% >> mixing residual)
HALFS = (43, 43, 42)
OFF = (0, 43, 86)
NJS = 48          # states j = 0..47 (init + 47 steps); 48 = 12 blocks of 4
NBLK = NJS // 4
SLOTW = 6 * 64    # stage slot width: 5 chunks * 64 + 64 zero pad
NCORES = 8
LN2 = float(np.log(2.0))
S0 = float(2.0 ** -10)   # A 2^6 * E 2^5 * s0 = 2 per step
HOST_EXACT = 3
XS = 40           # batch cols [0,XS) on DVE lane; [XS,64) on ACT->Pool lane
EW = NPC * 64     # 320: E'/R width per scan-step
F8 = ml_dtypes.float8_e4m3

_prog_cache = {}


def _softmax(x, axis=-1):
    x = x.astype(np.float64)
    m = x.max(axis=axis, keepdims=True)
    e = np.exp(x - m)
    return e / e.sum(axis=axis, keepdims=True)


def _build_program():
    import concourse.tile as tile
    from concourse import bacc, mybir
    from contextlib import ExitStack

    f8 = mybir.dt.float8e4
    f32 = mybir.dt.float32
    DR = mybir.MatmulPerfMode.DoubleRow
    MUL = mybir.AluOpType.mult
    YS = 64 - XS

    nc = bacc.Bacc(
        "TRN2",
        debug=False,
        enable_asserts=False,
        target_bir_lowering=False,
        num_devices=NCORES,
    )

    e8_d = nc.dram_tensor("e8", [PC, NBLK * NSCAN * 4 * EW], f8,
                          kind="ExternalInput").ap()
    st0_d = nc.dram_tensor("st0", [PC, NSCAN * EW], f8, kind="ExternalInput").ap()
    a_d = [
        nc.dram_tensor(f"a{kk}", [PC, 2 * MPAD], f8, kind="ExternalInput").ap()
        for kk in range(KKC)
    ]
    out_d = nc.dram_tensor(
        "out8", [NSCAN * NBLK * PC, 4 * SLOTW], f8, kind="ExternalOutput"
    ).ap()

    with tile.TileContext(nc) as tc:
        with ExitStack() as ctx:
            const = ctx.enter_context(tc.tile_pool(name="const", bufs=1))
            r8p = ctx.enter_context(tc.tile_pool(name="r8p", bufs=3))
            psp = ctx.enter_context(tc.tile_pool(name="psp", bufs=2, space="PSUM"))

            # ---- inputs ----
            a_sb = []
            for kk in range(KKC):
                t = const.tile([PC, 2 * MPAD], f8, tag=f"a{kk}", name=f"a{kk}")
                (nc.sync, nc.scalar, nc.sync)[kk].dma_start(t[:], a_d[kk][:])
                a_sb.append(t.rearrange("p (two m) -> p two m", two=2))

            e8all = const.tile([PC, NBLK * NSCAN * 4 * EW], f8, tag="e8",
                               name="e8all")
            EBLK = NSCAN * 4 * EW
            for k in range(NBLK):
                (nc.sync if k % 2 else nc.scalar).dma_start(
                    e8all[:, k * EBLK : (k + 1) * EBLK],
                    e8_d[:, k * EBLK : (k + 1) * EBLK],
                )
            e8v = e8all.rearrange(
                "p (blk s j4 pc b) -> p blk s j4 pc b", blk=NBLK, s=NSCAN, j4=4,
                b=64,
            )

            # ---- persistent stage supertiles (8 slots each) ----
            stages, stages_raw = [], []
            for s in range(NSCAN):
                st = const.tile([PC, 8 * SLOTW], f8, tag=f"stg{s}", name=f"stg{s}")
                sv = st.rearrange("p (sl pc b) -> p sl pc b", sl=8, b=64)
                # zero the pad chunk (pc=5) of every slot once
                nc.vector.memset(sv[:, :, 5, :], 0.0)
                stages.append(sv)
                stages_raw.append(st)
                # init: host-computed stage0 straight into slot 0
                nc.scalar.dma_start(
                    st[:, 0:EW], st0_d[:, s * EW : (s + 1) * EW]
                )

            def scan_mms(s, j):
                """R(s, j) = hat(s, j-1) @ A' -> PSUM [103, (pc, b)].
                Split into lane-A (cols 0:XS) and lane-B (XS:64) matmuls so
                each lane's consumers depend only on their own columns."""
                rp = psp.tile([PC, EW], f32, tag=f"r{s}", bufs=2,
                              name=f"r{s}_{j}")
                prev = stages[s][:, (j - 1) % 8]
                for pc in range(NPC):
                    for lo, hi in ((0, XS), (XS, 64)):
                        for kk in range(KKC):
                            nc.tensor.matmul(
                                rp[:, 64 * pc + lo : 64 * pc + hi],
                                lhsT=a_sb[kk][:, :, PC * pc : PC * (pc + 1)],
                                rhs=prev[:, 2 * kk : 2 * kk + 2, lo:hi],
                                start=(kk == 0),
                                stop=(kk == KKC - 1),
                                perf_mode=DR,
                            )
                return rp

            # ---- main rounds ----
            for j in range(1, NJS):
                for s in range(NSCAN):
                    rp = scan_mms(s, j)
                    rv = rp.rearrange("p (pc b) -> p pc b", b=64)
                    sl = j % 8
                    ev = e8v[:, j // 4, s, j % 4]
                    # lane A: DVE direct from PSUM
                    nc.vector.scalar_tensor_tensor(
                        stages[s][:, sl, 0:NPC, 0:XS], rv[:, :, 0:XS], S0,
                        ev[:, 0:XS], op0=MUL, op1=MUL,
                    )
                    # lane B: ACT scales R -> fp8 SBUF, Pool multiplies
                    r8 = r8p.tile([PC, NPC * YS], f8, tag=f"r8{s}", bufs=3,
                                  name=f"r8{s}_{j}")
                    nc.scalar.mul(r8[:], rv[:, :, XS:64], S0)
                    nc.gpsimd.scalar_tensor_tensor(
                        stages[s][:, sl, 0:NPC, XS:64],
                        r8.rearrange("p (pc b) -> p pc b", b=YS), 1.0,
                        ev[:, XS:64], op0=MUL, op1=MUL,
                    )
                    if j % 4 == 3:
                        k = j // 4
                        raw = stages_raw[s]
                        (nc.sync if (k + s) % 2 else nc.scalar).dma_start(
                            out_d[(s * NBLK + k) * PC : (s * NBLK + k + 1) * PC, :],
                            raw[:, (k % 2) * 4 * SLOTW : ((k % 2) * 4 + 4) * SLOTW],
                        )

    nc.compile()
    return nc


def _host_prep(inputs):
    obs = np.asarray(inputs["obs"], np.float32)
    A = _softmax(np.asarray(inputs["A_logits"]))          # [M, q, q] f64
    Bm = _softmax(np.asarray(inputs["B_logits"]))         # [M, q, D] f64
    pi = _softmax(np.asarray(inputs["init_logits"]))      # [M, q]    f64

    # normalized obs: E' = E / (2 * om);  om = mean_d obs  (per m, b, t)
    om = np.maximum(obs.mean(axis=-1, dtype=np.float64), 1e-6)  # [M, B, T]
    obs_n = obs / (2.0 * om[..., None]).astype(np.float32)

    # E'' = 32 * E' for all t, per model  [M, B, T, q] f32
    Ev = np.einsum("mbtd,mqd->mbtq", obs_n,
                   (Bm * 32.0).astype(np.float32), dtype=np.float32)

    A8 = np.zeros((M, Q, MPAD), F8)
    A8[:, :, :Q] = (A * 64.0).astype(F8)

    in_maps = []
    for core in range(NCORES):
        m, c = core // 4, core % 4
        e8 = np.zeros((PC, NBLK, NSCAN, 4, NPC, B), F8)
        st0 = np.zeros((PC, NSCAN, NPC, B), F8)
        for s in range(NSCAN):
            ts = np.clip(128 * c + OFF[s] + np.arange(NJS) - W, 0, T - 1)
            ee = Ev[m][:, ts, :]                        # [B, NJS, q] f32
            # [q, NJS, B] -> [pc, p, NJS, B]
            eq = ee.transpose(2, 1, 0).reshape(NPC, PC, NJS, B)
            e8[:, :, s] = eq.transpose(1, 2, 3, 0).reshape(
                PC, NBLK, 4, NPC, B).astype(F8)
            st0[:, s] = (
                eq[:, :, 0, :] * (pi[m].reshape(NPC, PC, 1) * 128.0)
            ).transpose(1, 0, 2).astype(F8)

        a_list = []
        for kk in range(KKC):
            a = np.zeros((PC, 2, MPAD), F8)
            for i in range(2):
                q0 = PC * (2 * kk + i)
                if q0 < Q:
                    a[:, i, :] = A8[m, q0 : q0 + PC, :]
            a_list.append(a)

        im = {
            "e8": e8.reshape(PC, NBLK * NSCAN * 4 * NPC * B),
            "st0": st0.reshape(PC, NSCAN * NPC * B),
        }
        for kk in range(KKC):
            im[f"a{kk}"] = a_list[kk].reshape(PC, 2 * MPAD)
        in_maps.append(im)

    return in_maps, (obs, om, A, Bm, pi)


def _host_assemble(results, obs, om, A, Bm, pi):
    # decode LUTs over fp8 bit patterns
    all8 = np.arange(256, dtype=np.uint8).view(F8).astype(np.float64)
    with np.errstate(divide="ignore", invalid="ignore"):
        loglut = np.log(all8)
    loglut[~np.isfinite(loglut)] = -80.0

    # cumulative obs-normalizer  C[m, b, t] = sum_{tau<=t} log(2*om)
    Cl = np.cumsum(np.log(2.0 * om), axis=-1)             # [M, B, T]

    out = np.empty((T, M, B, Q), np.float32)

    # exact alpha~0 sum for gamma seed: U~(0) = sum_q E'_0 * pi
    E0n = (obs[:, :, 0, :].astype(np.float64) / (2.0 * om[:, :, 0:1])) @ \
        Bm.transpose(0, 2, 1)                             # [M, B, q]
    true0 = np.log((E0n * pi[:, None, :]).sum(-1))        # [M, B]

    for m in range(M):
        lng = None
        prev_end = None
        for c in range(4):
            r = results[m * 4 + c]
            raw = np.asarray(r["out8"]).view(np.uint8).reshape(
                NSCAN, NBLK, PC, 4, 6, 64)[..., :5, :]    # [s, k, p, sl, pc, b]
            lin = all8[raw]                               # f64 linear values
            for s in range(NSCAN):
                jst, jen = W, W + HALFS[s]
                s_start = lin[s, jst // 4, :, jst % 4].sum((0, 1))  # [B]
                s_end = lin[s, jen // 4, :, jen % 4].sum((0, 1))    # [B]
                if c == 0 and s == 0:
                    lng = np.log(s_start) - true0[m] - W * LN2
                else:
                    hp = HALFS[s - 1] if s > 0 else HALFS[NSCAN - 1]
                    lng = lng + np.log(s_start) - np.log(prev_end) + hp * LN2
                prev_end = s_end
                lhat = loglut[raw[s]]                     # [k, p, sl, pc, b]
                for j in range(W, W + HALFS[s]):
                    t = 128 * c + OFF[s] + j - W
                    lh = lhat[j // 4, :, j % 4]           # [p, pc, b]
                    v = lh.transpose(2, 1, 0).reshape(B, Q)  # [b, q]
                    out[t, m] = (v - j * LN2 - lng[:, None]
                                 + Cl[m, :, t, None]).astype(np.float32)

    # exact first steps on host
    a = (E0n * pi[:, None, :]) * (2.0 * om[:, :, 0:1])    # true E0*pi
    ll = np.zeros((M, B, 1))
    for t in range(HOST_EXACT):
        S = a.sum(-1, keepdims=True)
        ll = ll + np.log(S)
        a = a / S
        out[t] = (np.log(a + 1e-32) + ll).astype(np.float32)
        Et1 = obs[:, :, t + 1, :].astype(np.float64) @ Bm.transpose(0, 2, 1)
        a = Et1 * np.einsum("mbq,mqp->mbp", a, A)
    return out


def kernel(**inputs) -> np.ndarray:
    from concourse import bass_utils

    in_maps, host_data = _host_prep(inputs)

    if "nc" not in _prog_cache:
        _prog_cache["nc"] = _build_program()
    nc = _prog_cache["nc"]

    res = bass_utils.run_bass_kernel_spmd(nc, in_maps, core_ids=list(range(NCORES)))
    return _host_assemble(res.results, *host_data)


if __name__ == "__main__":
    rng = np.random.default_rng(0)
    ins = {
        "obs": rng.random((M, B, T, D), np.float32),
        "A_logits": (rng.standard_normal((M, Q, Q)) * 0.1).astype(np.float32),
        "B_logits": (rng.standard_normal((M, Q, D)) * 0.1).astype(np.float32),
        "init_logits": (rng.standard_normal((M, Q)) * 0.1).astype(np.float32),
    }
    o = kernel(**ins)
    print("out", o.shape, o.dtype, np.isfinite(o).all())
